# revision 5
# baseline (speedup 1.0000x reference)
"""Single-head causal attention (B=8, T=2048, E=1024, H=64) on 8 trn2 cores.

Sharding: data-parallel over batch — one batch element per NeuronCore.

Per-core device kernel (all matmuls f32r unless KERNEL_FP32=1):
  inputs (host-prepped layouts):
    xT   [1024, 2048]  x_b transposed (E-major)  — avoids on-device transpose
    wq   [1024, 64]
    wkv  [1024, 128]   hstack(Wk, Wv)
    tril [128, 128]    lower-tri ones (f32) for diagonal-block causal mask
    eyeL [128, 64]     eyeL[64+r, r] = 1 — identity rows at partitions 64:128
    oneR [128, 64]     oneR[64, :] = 1  — ones row at partition 64
  output:
    outT [64, 2048]    attention output transposed; host transposes back

  Stage A (per i-group g of 512): qT[64,512]@p0 and (kT|vT)[128,512]
    accumulate over 8 E-chunks in PSUM; copy to SBUF.
  Stage B (per g): for each key block jb<=4g+3:
    ST[j,i] block = kT_jb.T @ qT_g   (scores transposed)
    exp via ScalarE with scale=1/8 folded in; causal: memset masked prefix,
    tril-multiply the diagonal 128-block
    accumulate O^T|sums = V_aug.T @ expST via PV matmul (V_aug has ones col)
  Normalize: recip(sums) broadcast via K=1 matmul; multiply; DMA out.
"""
import os
import sys
import numpy as np
from contextlib import ExitStack

sys.path.insert(0, "/opt/trn_rl_repo")

import concourse.bass as bass
import concourse.tile as tile
from concourse import bacc, mybir
from concourse.bass_utils import run_bass_kernel_spmd

F32 = mybir.dt.float32
F32R = mybir.dt.float32r

B, T, E, H = 8, 2048, 1024, 64
G = 4            # i-groups of 512 queries
GW = T // G      # 512
EC = E // 128    # 8 e-chunks
TB = T // 128    # 16 key blocks

_NC_CACHE = {}


def _mm_dtype():
    return F32 if os.environ.get("KERNEL_FP32") == "1" else F32R


def build_program():
    MMD = _mm_dtype()
    nc = bacc.Bacc("TRN2", target_bir_lowering=False, debug=False)
    xT_d = nc.dram_tensor("xT", [E, T], F32, kind="ExternalInput")
    wq_d = nc.dram_tensor("wq", [E, H], F32, kind="ExternalInput")
    wkv_d = nc.dram_tensor("wkv", [E, 2 * H], F32, kind="ExternalInput")
    tril_d = nc.dram_tensor("tril", [128, 128], F32, kind="ExternalInput")
    eyeL_d = nc.dram_tensor("eyeL", [128, H], F32, kind="ExternalInput")
    oneR_d = nc.dram_tensor("oneR", [128, H], F32, kind="ExternalInput")
    outT_d = nc.dram_tensor("outT", [H, T], F32, kind="ExternalOutput")

    with tile.TileContext(nc) as tc:
        with ExitStack() as ctx:
            build_body(ctx, tc, xT_d, wq_d, wkv_d, tril_d, eyeL_d, oneR_d,
                       outT_d, MMD)
    nc.compile()
    return nc


def build_body(ctx, tc, xT_d, wq_d, wkv_d, tril_d, eyeL_d, oneR_d, outT_d, MMD):
    nc = tc.nc

    const = ctx.enter_context(tc.tile_pool(name="const", bufs=1))
    qkv_sb = ctx.enter_context(tc.tile_pool(name="qkv_sb", bufs=1))
    vaug = ctx.enter_context(tc.tile_pool(name="vaug", bufs=1))
    xtp = ctx.enter_context(tc.tile_pool(name="xtp", bufs=3))
    expp = ctx.enter_context(tc.tile_pool(name="expp", bufs=3))
    outp = ctx.enter_context(tc.tile_pool(name="outp", bufs=2))
    smallp = ctx.enter_context(tc.tile_pool(name="smallp", bufs=2))

    ps_qk = ctx.enter_context(tc.tile_pool(name="ps_qk", bufs=2, space="PSUM"))
    ps_kv = ctx.enter_context(tc.tile_pool(name="ps_kv", bufs=1, space="PSUM"))
    ps_st = ctx.enter_context(tc.tile_pool(name="ps_st", bufs=2, space="PSUM"))
    ps_ot = ctx.enter_context(tc.tile_pool(name="ps_ot", bufs=1, space="PSUM"))
    ps_misc = ctx.enter_context(tc.tile_pool(name="ps_misc", bufs=1, space="PSUM"))

    # ---- constants / weights ----
    wq_sb = const.tile([128, EC, H], MMD)
    nc.sync.dma_start(
        out=wq_sb,
        in_=wq_d.ap().rearrange("(ec p) h -> p ec h", p=128).bitcast(MMD),
    )
    wkv_sb = const.tile([128, EC, 2 * H], MMD)
    nc.sync.dma_start(
        out=wkv_sb,
        in_=wkv_d.ap().rearrange("(ec p) h -> p ec h", p=128).bitcast(MMD),
    )
    tril_sb = const.tile([128, 128], MMD)
    nc.sync.dma_start(out=tril_sb, in_=tril_d.ap().bitcast(MMD))
    eyeL_sb = const.tile([128, H], MMD)
    nc.sync.dma_start(out=eyeL_sb, in_=eyeL_d.ap().bitcast(MMD))
    oneR_sb = const.tile([128, H], MMD)
    nc.sync.dma_start(out=oneR_sb, in_=oneR_d.ap().bitcast(MMD))

    # persistent SBUF state
    qT_sb = [qkv_sb.tile([H, GW], MMD, name=f"qT{g}") for g in range(G)]
    kv_sb = [qkv_sb.tile([128, GW], MMD, name=f"kv{g}") for g in range(G)]
    va_sb = [vaug.tile([128, H + 1], MMD, name=f"va{jb}") for jb in range(TB)]

    for g in range(G):
        # ---------------- Stage A: projections for this i/key group -------
        qk_ps = ps_qk.tile([H, GW], F32, name="q_ps")
        kv_ps = ps_kv.tile([128, GW], F32, name="kv_ps")
        for ec in range(EC):
            xt = xtp.tile([128, GW], MMD, name="xt")
            nc.sync.dma_start(
                out=xt,
                in_=xT_d.ap()[ec * 128:(ec + 1) * 128,
                              g * GW:(g + 1) * GW].bitcast(MMD),
            )
            nc.tensor.matmul(
                qk_ps, lhsT=wq_sb[:, ec, :], rhs=xt[:],
                start=(ec == 0), stop=(ec == EC - 1),
            )
            nc.tensor.matmul(
                kv_ps, lhsT=wkv_sb[:, ec, :], rhs=xt[:],
                start=(ec == 0), stop=(ec == EC - 1),
            )
        nc.vector.tensor_copy(qT_sb[g][:], qk_ps[:])
        nc.vector.tensor_copy(kv_sb[g][:], kv_ps[:])

        # V_aug blocks for this group's 4 key blocks: transpose vT -> V
        for m in range(4):
            jb = 4 * g + m
            tr_ps = ps_misc.tile([128, H], F32, name="tr_ps")
            # vT block [64@p64, 128] -> V block [128, 64]
            nc.tensor.matmul(
                tr_ps, lhsT=kv_sb[g][H:128, m * 128:(m + 1) * 128],
                rhs=eyeL_sb[H:128, :], start=True, stop=True,
            )
            nc.vector.tensor_copy(va_sb[jb][:, 0:H], tr_ps[:])
            nc.vector.memset(va_sb[jb][:, H:H + 1].bitcast(F32), 1.0)

        # ---------------- Stage B: scores/softmax/PV for group g ----------
        ot_ps = ps_ot.tile([H + 1, GW], F32, name="ot_ps")
        njb = 4 * g + 4  # key blocks 0 .. 4g+3
        for jb in range(njb):
            st_ps = ps_st.tile([128, GW], F32, name="st_ps")
            nc.tensor.matmul(
                st_ps,
                lhsT=kv_sb[jb // 4][0:H, (jb % 4) * 128:(jb % 4 + 1) * 128],
                rhs=qT_sb[g][:],
                start=True, stop=True,
            )
            ex = expp.tile([128, GW], MMD, name="ex")
            off = jb * 128 - g * GW  # >0 only for diagonal blocks past group start
            if off > 0:
                nc.vector.memset(ex[:, 0:off].bitcast(F32), 0.0)
                nc.scalar.activation(
                    out=ex[:, off:GW], in_=st_ps[:, off:GW],
                    func=mybir.ActivationFunctionType.Exp, scale=0.125,
                )
            else:
                nc.scalar.activation(
                    out=ex[:], in_=st_ps[:],
                    func=mybir.ActivationFunctionType.Exp, scale=0.125,
                )
            if jb >= 4 * g:  # diagonal block: apply lower-tri mask
                doff = max(off, 0)
                nc.vector.tensor_mul(
                    ex[:, doff:doff + 128], ex[:, doff:doff + 128], tril_sb[:]
                )
            nc.tensor.matmul(
                ot_ps, lhsT=va_sb[jb][:], rhs=ex[:],
                start=(jb == 0), stop=(jb == njb - 1),
            )

        # ---------------- normalize + store -------------------------------
        rec = smallp.tile([128, GW], MMD, name="rec")
        with nc.allow_low_precision(reason="f32r feed for broadcast matmul"):
            nc.vector.reciprocal(rec[H:H + 1, :], ot_ps[H:H + 1, :])
        rb_ps = ps_misc.tile([H, GW], F32, name="rb_ps")
        nc.tensor.matmul(
            rb_ps, lhsT=oneR_sb[H:H + 1, 0:H], rhs=rec[H:H + 1, :],
            start=True, stop=True,
        )
        rb_sb = smallp.tile([H, GW], F32, name="rb_sb")
        nc.vector.tensor_copy(rb_sb[:], rb_ps[:])
        o_sb = outp.tile([H, GW], F32, name="o_sb")
        nc.vector.tensor_mul(o_sb[:], ot_ps[0:H, :], rb_sb[:])
        nc.sync.dma_start(out=outT_d.ap()[:, g * GW:(g + 1) * GW], in_=o_sb[:])


def _get_program():
    key = _mm_dtype()
    if key not in _NC_CACHE:
        _NC_CACHE[key] = build_program()
    return _NC_CACHE[key]


def kernel(x, Wq, Wk, Wv, **run_kwargs):
    x = np.asarray(x, dtype=np.float32)
    Wq = np.asarray(Wq, dtype=np.float32)
    Wk = np.asarray(Wk, dtype=np.float32)
    Wv = np.asarray(Wv, dtype=np.float32)
    assert x.shape == (B, T, E)

    xT = np.ascontiguousarray(x.transpose(0, 2, 1))          # [B, E, T]
    wkv = np.ascontiguousarray(np.concatenate([Wk, Wv], axis=1))  # [E, 128]
    tril = np.triu(np.ones((128, 128), dtype=np.float32))
    eyeL = np.zeros((128, H), dtype=np.float32)
    eyeL[H:128, :] = np.eye(H, dtype=np.float32)
    oneR = np.zeros((128, H), dtype=np.float32)
    oneR[H, :] = 1.0

    nc = _get_program()
    in_maps = [
        {"xT": xT[b], "wq": Wq, "wkv": wkv, "tril": tril,
         "eyeL": eyeL, "oneR": oneR}
        for b in range(B)
    ]
    res = run_bass_kernel_spmd(nc, in_maps, core_ids=list(range(B)),
                               **run_kwargs)
    out = np.stack([r["outT"].T for r in res.results])       # [B, T, H]
    if run_kwargs:
        kernel.last_results = res
    return out


if __name__ == "__main__":
    rng = np.random.default_rng(0)
    x = rng.standard_normal((B, T, E), dtype=np.float32)
    s = 1.0 / np.sqrt(E)
    Wq = rng.standard_normal((E, H), dtype=np.float32) * s
    Wk = rng.standard_normal((E, H), dtype=np.float32) * s
    Wv = rng.standard_normal((E, H), dtype=np.float32) * s
    out = kernel(x=x, Wq=Wq, Wk=Wk, Wv=Wv)
    print("out", out.shape, out.dtype, np.abs(out).max())
